# revision 1
# baseline (speedup 1.0000x reference)
"""Multi-head self-attention on 8 TRN2 NeuronCores.

Problem: x(4,2048,1024), Wq(8,1024,128), Wk/Wv(1024,128), Wo(1024,1024) fp32.
out = softmax(Q K^T / sqrt(128)) V -> concat heads -> @ Wo.

Sharding: (batch, query-half) across 8 cores — core c handles batch c//2,
query rows [(c%2)*1024, (c%2)*1024+1024). K/V cover the full sequence of the
batch, so each core computes them locally from its x slice; no collectives.

Numerics: scores have std ~1024 and softmax is near-one-hot, so the
x->Q/K->scores chain needs ~12+ mantissa bits. The tensor engine's float32r
mode delivers ~12-13 effective mantissa bits at 1 cycle/row for >=256-wide
moving operands (4x faster than plain fp32, equal to bf16), so x/Wq/Wk/Wv
stay fp32 (dtype float32r end-to-end) and every projection plus the scores
matmul is a single f32r pass instead of three bf16 split passes. The
P/ctx/Wo path runs fp16 (better than bf16 at the same speed).

Layouts (partition dim first): xT (E,S) host-transposed; K^T (O,S) =
Wk-stationary @ xT; Q_h^T (O,Sq) likewise (Wq pre-scaled by 1/sqrt(O));
scores tile (128q, 2048s) = Q^T-slice-stationary @ K^T-moving (f32r, fp32
PSUM, two 1024-halves). Softmax per q-row: per-half DVE reduce_max; half 0
exps immediately against its OWN max (keeps the ACT exp stream off the
half-1 critical path), half 1 against the global max; ACT accumulates the
dens; den = f0*den0 + den1 with f0 = exp(m0 - mg). The per-half
normalization factors are replicated 128-wide (ACT Identity, per-partition
scale), PE-transposed into row-broadcast form, and multiplied in during the
P^T PSUM->SBUF copies (DVE tensor_tensor with a stride-0 broadcast operand)
- the normalization costs no dedicated pass over P. P^T transposes run on
the PE in two 8-s-tile waves lagged two q-tiles behind the softmax so every
PE instruction is long-ready (stalls also drop the PE out of max p-state);
ctx^T (O,Sq) = V-stationary @ P^T-moving; out (Sq,E) = ctx-slices-stationary
@ Wo-moving. Weights are host-pre-permuted to partition-major so all DMAs
are contiguous 2D copies spread over the three issue queues.
"""
import numpy as np

B, S, E, H, O = 4, 2048, 1024, 8, 128
SQ = S // 2          # query rows per core
NCORES = 8
ET = E // 128        # 8 e-tiles
ST = S // 128        # 16 s-tiles
QT = SQ // 128       # 8 q-tiles
NB = S // 512        # 4 score banks per q-tile
EC = E // 512        # 2 out-proj column chunks

_compiled = None     # cache so repeated kernel() calls skip rebuild


def _build():
    import concourse.bass as bass
    import concourse.mybir as mybir
    import concourse.tile as tile
    from concourse import bacc
    from concourse.masks import make_identity

    F32 = mybir.dt.float32
    F32R = mybir.dt.float32r
    FP16 = mybir.dt.float16
    PS = bass.MemorySpace.PSUM
    EXP = mybir.ActivationFunctionType.Exp

    nc = bacc.Bacc("TRN2", target_bir_lowering=False, debug=False,
                   enable_asserts=True)

    # xkv columns are pre-permuted per core so its query half is always
    # columns [0, SQ) — attention is permutation-invariant over the key axis,
    # so the same NEFF slices queries identically on every core.
    d_xkv = nc.dram_tensor("xkv", (E, S), F32R, kind="ExternalInput").ap()
    d_wq = nc.dram_tensor("wq", (H, 128, ET * O), F32R, kind="ExternalInput").ap()
    d_wk = nc.dram_tensor("wk", (128, ET * O), F32R, kind="ExternalInput").ap()
    d_wv = nc.dram_tensor("wv", (128, ET * O), F32R, kind="ExternalInput").ap()
    d_wo = nc.dram_tensor("wo", (128, H * E), FP16, kind="ExternalInput").ap()
    d_out = nc.dram_tensor("out", (SQ, E), F32, kind="ExternalOutput").ap()

    with tile.TileContext(nc) as tc:
        with (
            tc.tile_pool(name="persist", bufs=1) as persist,
            tc.tile_pool(name="tiny", bufs=24) as tiny,
        ):
            ident = persist.tile([128, 128], FP16, tag="ident")

            # DMA queue = issuing engine; spread big loads across queues
            # (sync/scalar are HWDGE, gpsimd SWDGE) — a single queue
            # streams only ~38 GB/s
            wo_sb = persist.tile([128, H, E], FP16, tag="wo")

            kt = persist.tile([128, S], F32R, tag="kt")
            qt_sb = persist.tile([128, H, SQ], F32R, tag="qt")
            v_sb = persist.tile([128, ST, O], FP16, tag="v")

            # ---------------- prologue: K^T, V, Q^T projections ----------
            # DMA is the prologue bottleneck (~14.5 MB in): spread across all
            # four issue queues and order chunks by first use; Q-proj runs in
            # two 4-head PSUM blocks with the e-loop OUTER so the PE starts
            # on the first arriving e-tile instead of waiting for all of x.
            with tc.tile_pool(name="xp", bufs=1) as xp:
                wk = xp.tile([128, ET, O], F32R, tag="wk")
                wv = xp.tile([128, ET, O], F32R, tag="wv")
                xkv = xp.tile([128, ET, S], F32R, tag="xkv")
                wq = xp.tile([128, H, ET, O], F32R, tag="wq")
                # The Q phase is gated on the query-half of x plus wq[h]
                # just before head h: spread the x query-half over all three
                # queues so it lands in ~10us, interleave wq h0-3 between
                # the scalar queue's share, and stream the kv-only half
                # behind it in K-phase e order.
                def qx(e):
                    return (xkv[:, e, 0:SQ], d_xkv[e * 128:(e + 1) * 128, 0:SQ])
                def kx(e):
                    return (xkv[:, e, SQ:S], d_xkv[e * 128:(e + 1) * 128, SQ:S])
                def qw(h):
                    return (wq[:, h, :, :].rearrange("p t o -> p (t o)"),
                            d_wq[h])
                nc.scalar.dma_start(*qw(0))
                nc.scalar.dma_start(*qx(6))
                nc.scalar.dma_start(*qx(7))
                nc.scalar.dma_start(*qw(1))
                nc.scalar.dma_start(*qw(2))
                nc.scalar.dma_start(*qw(3))
                nc.scalar.dma_start(
                    wo_sb[:].rearrange("p h e -> p (h e)"), d_wo)
                for e in range(3):
                    nc.gpsimd.dma_start(*qx(e))
                for h in range(4, 8):
                    nc.gpsimd.dma_start(*qw(h))
                for e in range(6, ET):
                    nc.gpsimd.dma_start(*kx(e))
                for e in range(3, 6):
                    nc.sync.dma_start(*qx(e))
                nc.sync.dma_start(wk[:].rearrange("p t o -> p (t o)"), d_wk)
                nc.sync.dma_start(wv[:].rearrange("p t o -> p (t o)"), d_wv)
                for e in range(6):
                    nc.sync.dma_start(*kx(e))
                make_identity(nc, ident[:])

                # Q^T per head (head h gated only on wq[h] and the
                # arriving query-half x tiles)
                with tc.tile_pool(name="qp", bufs=3, space=PS) as qp:
                    for h in range(H):
                        q_ps = qp.tile([128, SQ], F32, tag="qtps")
                        for e in range(ET):
                            for c in range(SQ // 512):
                                nc.tensor.matmul(
                                    q_ps[:, c * 512:(c + 1) * 512],
                                    wq[:, h, e, :],
                                    xkv[:, e, c * 512:(c + 1) * 512],
                                    start=(e == 0),
                                    stop=(e == ET - 1),
                                )
                        nc.scalar.copy(qt_sb[:, h, :], q_ps[:])

                # K^T and V^T share one PSUM scope (4+4 banks) so their
                # matmuls interleave and neither phase-transition stalls PE
                with (
                    tc.tile_pool(name="ktp", bufs=1, space=PS) as ktp,
                    tc.tile_pool(name="vtp", bufs=1, space=PS) as vtp,
                ):
                    kt_ps = ktp.tile([128, S], F32, tag="kt")
                    vt_ps = vtp.tile([128, S], F32, tag="vt")
                    for e in range(ET):
                        for c in range(NB):
                            nc.tensor.matmul(
                                kt_ps[:, c * 512:(c + 1) * 512],
                                wk[:, e, :],
                                xkv[:, e, c * 512:(c + 1) * 512],
                                start=(e == 0),
                                stop=(e == ET - 1),
                            )
                        for c in range(NB):
                            nc.tensor.matmul(
                                vt_ps[:, c * 512:(c + 1) * 512],
                                wv[:, e, :],
                                xkv[:, e, c * 512:(c + 1) * 512],
                                start=(e == 0),
                                stop=(e == ET - 1),
                            )
                    nc.scalar.copy(kt[:], kt_ps[:])
                    vt_sb = xp.tile([128, S], FP16, tag="vtsb")
                    nc.scalar.copy(vt_sb[:], vt_ps[:])
                with tc.tile_pool(name="vsp", bufs=2, space=PS) as vsp:
                    for g in range(2):
                        v_st = vsp.tile([128, 8, 128], FP16, tag="vst")
                        for k in range(8):
                            st = g * 8 + k
                            nc.tensor.transpose(
                                v_st[:, k, :],
                                vt_sb[:, st * 128:(st + 1) * 128],
                                ident[:],
                            )
                        nc.vector.tensor_copy(
                            v_sb[:, g * 8:(g + 1) * 8, :], v_st[:])

            # ---------------- main: per-head attention ------------------
            # PSUM budget (8 banks, 16 KiB/partition): "acc1024" 4KB tiles x3
            # bufs shared by score-halves, ctx and out accumulators (12KB) +
            # one 2KB transpose-wave tile + one 256B invden-transpose tile.
            # P^T transposes run in two 8-s-tile waves lagged one q-tile so
            # each wave's PSUM->SBUF copy hides under the next scores matmul
            # even with a single wave buffer.
            ones128 = persist.tile([128, 128], FP16, tag="ones")
            nc.vector.memset(ones128[:], 1.0)
            with (
                tc.tile_pool(name="p_pool", bufs=4) as p_pool,
                tc.tile_pool(name="pt_pool", bufs=2) as pt_pool,
                tc.tile_pool(name="ctx_pool", bufs=H) as ctx_pool,
                tc.tile_pool(name="acc_ps", bufs=3, space=PS) as acc_psp,
                tc.tile_pool(name="pt_ps", bufs=1, space=PS) as pt_psp,
                tc.tile_pool(name="sm_ps", bufs=1, space=PS) as sm_psp,
                tc.tile_pool(name="o_sb", bufs=2) as o_sbp,
            ):
                HS = S // 2  # 1024-wide score half
                MIN = mybir.AluOpType.min
                MUL = mybir.AluOpType.mult
                IDN = mybir.ActivationFunctionType.Identity

                def emit_invrepT(pend):
                    # PE-transpose the column-replicated normalization tiles:
                    # the result has the factor for q in every PARTITION at
                    # free pos q — the row-broadcast operands the fused P^T
                    # copies need (half 0 carries the extra exp(m0-mg) fixup)
                    sm = sm_psp.tile([128, 2, 128], FP16, tag="sm")
                    nc.tensor.transpose(sm[:, 0, :], pend["invrepA"][:],
                                        ident[:])
                    nc.tensor.transpose(sm[:, 1, :], pend["invrepB"][:],
                                        ident[:])
                    bcA = tiny.tile([128, 128], FP16, tag="bcastA")
                    nc.scalar.copy(bcA[:], sm[:, 0, :])
                    bcB = tiny.tile([128, 128], FP16, tag="bcastB")
                    nc.scalar.copy(bcB[:], sm[:, 1, :])
                    pend["bcast"] = (bcA, bcB)

                def emit_wave(pend, w):
                    # 8 PE transposes of p' 128x128 blocks, then one fused
                    # DVE copy PSUM->SBUF that multiplies by invden[q]
                    # (broadcast along partitions and s-tiles): the softmax
                    # normalization rides the copy for free.
                    p_qt, pt_h, qt = pend["p"], pend["pt"], pend["qt"]
                    pt_ps = pt_psp.tile([128, 8, 128], FP16, tag="ptps")
                    for k in range(8):
                        st = w * 8 + k
                        nc.tensor.transpose(
                            pt_ps[:, k, :],
                            p_qt[:, st * 128:(st + 1) * 128],
                            ident[:],
                        )
                    nc.vector.tensor_tensor(
                        out=pt_h[:, w * 8:(w + 1) * 8,
                                 qt * 128:(qt + 1) * 128],
                        in0=pt_ps[:],
                        in1=pend["bcast"][w][:].unsqueeze(1)
                        .to_broadcast([128, 8, 128]),
                        op=MUL,
                    )

                ctxs = []

                def emit_ctx_half(state, qc):
                    # ctx^T (o-part, q-free) accumulated over s-tiles; lagged
                    # into the next head's score phase as PE filler, one
                    # 512-wide half-burst at a time to limit the disruption
                    pt_h = state["pt"]
                    ctx_h = state["ctx"]
                    if state["ct"] is None:
                        ct_ps = acc_psp.tile([128, SQ], F32, tag="acc1024")
                        state["ct"] = ct_ps
                    ct_ps = state["ct"]
                    for st in range(ST):
                        nc.tensor.matmul(
                            ct_ps[:, qc * 512:(qc + 1) * 512],
                            v_sb[:, st, :],
                            pt_h[:, st, qc * 512:(qc + 1) * 512],
                            start=(st == 0),
                            stop=(st == ST - 1),
                        )
                    nc.scalar.copy(
                        ctx_h[:, qc * 512:(qc + 1) * 512],
                        ct_ps[:, qc * 512:(qc + 1) * 512])

                pend = None
                pend2 = None
                pending_ctx = None
                pt_h = None
                for t in range(H * QT):
                    h, qt = divmod(t, QT)
                    if qt == 0:
                        if h > 0:
                            ctx_h = ctx_pool.tile([128, SQ], FP16, tag="ctx")
                            pending_ctx = {"pt": pt_h, "ct": None,
                                           "ctx": ctx_h}
                            ctxs.append(ctx_h)
                        pt_h = pt_pool.tile([128, ST, SQ], FP16, tag="pt")

                    nm2 = tiny.tile([128, 2], F32, tag="nm2")
                    den2 = tiny.tile([128, 2], F32, tag="den2")
                    p_qt = p_pool.tile([128, S], FP16, tag="p")
                    s_half = []
                    for sh in range(2):
                        s_ps = acc_psp.tile([128, HS], F32, tag="acc1024")
                        s_half.append(s_ps)
                        for c in range(2):
                            nc.tensor.matmul(
                                s_ps[:, c * 512:(c + 1) * 512],
                                qt_sb[:, h, qt * 128:(qt + 1) * 128],
                                kt[:, sh * HS + c * 512:
                                   sh * HS + (c + 1) * 512],
                                start=True,
                                stop=True,
                            )
                        nc.vector.reduce_max(
                            out=nm2[:, sh:sh + 1], in_=s_ps[:],
                            axis=mybir.AxisListType.X, negate=True,
                        )
                        if sh == 0:
                            # half 0 exps against its OWN max so the exp
                            # stream never waits on half 1's matmul+max:
                            # shortens the per-q-tile critical path by ~2us
                            nc.scalar.activation(
                                p_qt[:, 0:HS], s_ps[:],
                                EXP, bias=nm2[:, 0:1], scale=1.0,
                                accum_out=den2[:, 0:1],
                            )
                        if sh == 0 and pend2 is not None:
                            # PE filler between the score halves, lagged TWO
                            # q-tiles so the invden chain (exp->den->recip->
                            # replica) is long resolved and the PE never
                            # stalls (stalls also drop the PE out of its max
                            # p-state, slowing every matmul)
                            emit_invrepT(pend2)
                            emit_wave(pend2, 0)
                    nmg = tiny.tile([128, 1], F32, tag="nmg")
                    nc.vector.tensor_reduce(
                        out=nmg[:], in_=nm2[:],
                        axis=mybir.AxisListType.X, op=MIN,
                    )
                    nc.scalar.activation(
                        p_qt[:, HS:S], s_half[1][:],
                        EXP, bias=nmg[:], scale=1.0,
                        accum_out=den2[:, 1:2],
                    )
                    # f0 = exp(m0 - mg) rescales half 0 (which exp'd against
                    # its own max) onto the global-max scale; den =
                    # f0*den0 + den1; the per-half normalization replicas
                    # ride ACT Identity ops with per-partition AP scales
                    f0 = tiny.tile([128, 1], F32, tag="f0")
                    nc.scalar.activation(
                        f0[:], nm2[:, 0:1], EXP, bias=nmg[:], scale=-1.0)
                    den = tiny.tile([128, 1], F32, tag="den")
                    nc.vector.scalar_tensor_tensor(
                        out=den[:], in0=den2[:, 0:1], scalar=f0[:],
                        in1=den2[:, 1:2], op0=MUL,
                        op1=mybir.AluOpType.add)
                    invden = tiny.tile([128, 1], F32, tag="invden")
                    nc.vector.reciprocal(invden[:], den[:])
                    sclA = tiny.tile([128, 1], F32, tag="sclA")
                    nc.vector.tensor_mul(sclA[:], f0[:], invden[:])
                    invrepA = tiny.tile([128, 128], FP16, tag="invrepA")
                    nc.scalar.activation(
                        invrepA[:], ones128[:], IDN, scale=sclA[:])
                    invrepB = tiny.tile([128, 128], FP16, tag="invrepB")
                    nc.scalar.activation(
                        invrepB[:], ones128[:], IDN, scale=invden[:])

                    if pending_ctx is not None and qt in (1, 3):
                        emit_ctx_half(pending_ctx, qt // 2)
                        if qt == 3:
                            pending_ctx = None
                    if pend2 is not None:
                        emit_wave(pend2, 1)
                    pend2 = pend
                    pend = {"p": p_qt, "pt": pt_h, "qt": qt,
                            "invrepA": invrepA, "invrepB": invrepB}
                for last in (pend2, pend):
                    emit_invrepT(last)
                    emit_wave(last, 0)
                    emit_wave(last, 1)
                ctx_h = ctx_pool.tile([128, SQ], FP16, tag="ctx")
                pending_ctx = {"pt": pt_h, "ct": None, "ctx": ctx_h}
                ctxs.append(ctx_h)
                for qc in range(2):
                    emit_ctx_half(pending_ctx, qc)

                # ------- out (q-part, e-free) = sum_h ctx_h^T-slices @ Wo_h
                for qt in range(QT):
                    o_ps = acc_psp.tile([128, E], F32, tag="acc1024")
                    for h in range(H):
                        for ec in range(EC):
                            nc.tensor.matmul(
                                o_ps[:, ec * 512:(ec + 1) * 512],
                                ctxs[h][:, qt * 128:(qt + 1) * 128],
                                wo_sb[:, h, ec * 512:(ec + 1) * 512],
                                start=(h == 0),
                                stop=(h == H - 1),
                            )
                    o_sb = o_sbp.tile([128, E], F32, tag="osb")
                    nc.scalar.copy(o_sb[:], o_ps[:])
                    dmae = nc.gpsimd if qt % 2 else nc.sync
                    dmae.dma_start(
                        d_out[qt * 128:(qt + 1) * 128, :], o_sb[:])

    nc.compile()
    return nc


def prep_inputs(x, Wq, Wk, Wv, Wo):
    scale = np.float32(1.0 / np.sqrt(O))

    def perm(w):  # (T*128, N) -> (128, T*N): partition-major tiles
        t = w.shape[0] // 128
        return np.ascontiguousarray(
            w.reshape(t, 128, -1).transpose(1, 0, 2).reshape(128, -1))

    wq = np.stack([perm(Wq[h].astype(np.float32) * scale) for h in range(H)])
    wk = perm(Wk.astype(np.float32))
    wv = perm(Wv.astype(np.float32))
    wo = perm(Wo.astype(np.float16))

    in_maps = []
    xts = {}
    for b in range(B):
        xts[b] = np.ascontiguousarray(x[b].T.astype(np.float32))  # (E, S)
    for c in range(NCORES):
        b, half = divmod(c, 2)
        xt = xts[b]
        if half == 1:
            # rotate so this core's query half occupies columns [0, SQ);
            # attention is permutation-invariant over the key/value axis
            xt = np.ascontiguousarray(np.roll(xt, SQ, axis=1))
        in_maps.append({
            "xkv": xt, "wq": wq, "wk": wk, "wv": wv, "wo": wo,
        })
    return in_maps


def kernel(x, Wq, Wk, Wv, Wo):
    global _compiled
    from concourse.bass_utils import run_bass_kernel_spmd

    x = np.asarray(x, dtype=np.float32)
    Wq = np.asarray(Wq, dtype=np.float32)
    Wk = np.asarray(Wk, dtype=np.float32)
    Wv = np.asarray(Wv, dtype=np.float32)
    Wo = np.asarray(Wo, dtype=np.float32)

    if _compiled is None:
        _compiled = _build()
    nc = _compiled

    in_maps = prep_inputs(x, Wq, Wk, Wv, Wo)

    res = run_bass_kernel_spmd(nc, in_maps, core_ids=list(range(NCORES)))

    out = np.empty((B, S, E), dtype=np.float32)
    for c in range(NCORES):
        b, half = divmod(c, 2)
        out[b, half * SQ:(half + 1) * SQ, :] = res.results[c]["out"]
    return out

